# revision 15
# baseline (speedup 1.0000x reference)
"""Trainium2 Bass kernel for ContrastiveNet loss (v5: host-normalized rows,
chunk-major gram pipeline, HBM scatter planes, compressed last-chunk tail).

Algorithm (per core k of 8, SPMD):
  - host: xn = x / ||x|| * S (S=32), cast fp8e4 (so sim = G / (S^2*TEMP) with
    G the raw fp8 gram; no on-device normalization). Rows rolled so core k's
    512 anchor rows sit at rotated columns 0..511. y laid out COLUMN-CHUNK-
    major: [8 chunks][128 part][16 kt][512 cols] so each chunk's gram (4 row-
    tiles x 8 kp DoubleRow matmuls) runs as soon as the chunk lands.
  - DMA order puts chunk 7 dead last; everything that does not depend on it
    (plane pieces, dup-pass planes, pair masks) lands earlier, so the post-
    load tail is only: gram(c7) + drain + 4 narrow 512-scan scatters + a
    narrow late dup pass + exp/den/lse.
  - gather: per (tile, piece) gpsimd local_scatter with HBM col->slot planes
    (int16, -1 unused; pieces 1024,1024,1024,512,512). Values accumulate
    in-place per tile. Duplicate (row,col) refs (2-member groups) fixed by
    scatter passes: early pass (full NE, runs before chunk 7) for groups
    whose first col is in pieces 0-3; late pass (narrow: such pairs are
    ranked first) for groups entirely inside piece 4.
  - pairs touching >=3-member dup groups (~2%) are masked out and computed
    exactly on host; total = (device masked sum + host sum) / P.
  - PE p-state: dummy matmuls bridge the DMA-bound idle before chunk 7 so
    its gram runs at full clock.
"""
import os
import sys
import numpy as np
import ml_dtypes

try:
    import concourse  # noqa: F401
except ImportError:
    sys.path.insert(0, "/opt/trn_rl_repo")

from contextlib import ExitStack

import concourse.bass as bass
import concourse.tile as tile
from concourse import bacc, mybir
from concourse._compat import with_exitstack
from concourse.bass_utils import run_bass_kernel_spmd

F16 = np.float16
FP8 = ml_dtypes.float8_e4m3
F32 = mybir.dt.float32
DF16 = mybir.dt.float16
F8 = mybir.dt.float8e4
I16 = mybir.dt.int16

B, D, J = 4096, 2048, 11
NCORES, RPC, NT, NKP = 8, 512, 4, 8
NCH, CW = 8, 512                    # gram column chunks (per core)
POFF = [0, 1024, 2048, 3072, 3584]  # scatter piece offsets
PW = [1024, 1024, 1024, 512, 512]   # scatter piece widths
NP = 5
TEMP = 0.1
S = 32.0
KSC = 1.0 / (S * S * TEMP)
NWARM = 52                          # PE p-state bridge matmuls before chunk 7
AF = mybir.ActivationFunctionType
ALU = mybir.AluOpType
AX = mybir.AxisListType
DR = mybir.MatmulPerfMode.DoubleRow


def _even(n):
    return n + (n % 2)


# ---------------------------------------------------------------- host prep
def build_plan(anchor_idx, pos_idx, neg_idx):
    r = anchor_idx.astype(np.int64)
    cols = np.concatenate([pos_idx[:, None], neg_idx], axis=1).astype(np.int64)
    P = r.shape[0]

    # ---- duplicate groups over (row, col)
    er = np.repeat(r, J)
    ec = cols.ravel()
    pair_of = np.repeat(np.arange(P), J)
    core = er // RPC
    t = (er % RPC) // 128
    pp = er % 128
    ec_rot = (ec - core * RPC) % B
    piece = np.searchsorted(POFF, ec_rot, side="right") - 1

    key = er * B + ec
    o2 = np.argsort(key, kind="stable")
    k_sorted = key[o2]
    first2 = np.r_[True, k_sorted[1:] != k_sorted[:-1]]
    gid_sorted = np.cumsum(first2) - 1
    NG = int(gid_sorted[-1]) + 1
    gid = np.empty(P * J, np.int64)
    gid[o2] = gid_sorted
    gsz_g = np.bincount(gid_sorted, minlength=NG)
    gsz = gsz_g[gid]

    # pairs containing any entry of a >=3-member group -> host, masked out
    bad_pairs = np.unique(pair_of[gsz >= 3])
    badp = np.zeros(P, bool)
    badp[bad_pairs] = True

    # 2-member groups: rep = member in the earliest piece
    two = gsz == 2
    order = np.lexsort((np.arange(P * J), piece, gid))
    go = order[two[order]]    # 2-group members, grouped by gid, piece-sorted
    g_of_go = gid[go]
    firstg = np.r_[True, g_of_go[1:] != g_of_go[:-1]]
    rep = go[firstg]          # occ0 entry per 2-group
    oth = go[~firstg]         # occ1 entry per 2-group (same group order)
    rep_bad = badp[pair_of[rep]]
    oth_bad = badp[pair_of[oth]]
    # rep in a bad pair but partner not: partner becomes the plane entry
    swap = rep_bad & ~oth_bad
    # device dup pass only when both members' pairs are live
    pk = ~rep_bad & ~oth_bad
    rep_p, oth_p = rep[pk], oth[pk]
    late_g = piece[rep_p] == NP - 1        # both members in last piece

    # ---- pair ranking: pairs touching late groups first (narrow late pass)
    late_pairs = np.unique(np.r_[pair_of[rep_p[late_g]],
                                 pair_of[oth_p[late_g]]])
    sev = np.zeros(P, np.int64)
    sev[late_pairs] = 1
    order_p = np.lexsort((np.arange(P), -sev, r))
    r_sp = r[order_p]
    firstp = np.r_[True, r_sp[1:] != r_sp[:-1]]
    gidp = np.cumsum(firstp) - 1
    rank_sorted = np.arange(P) - np.flatnonzero(firstp)[gidp]
    srank = np.empty(P, np.int64)
    srank[order_p] = rank_sorted

    n_per_row = np.bincount(r, minlength=B)
    SP = int(max(n_per_row.max(), 1))
    NE = _even(SP * J)
    assert NE * 32 < 2**16
    eslot = srank[pair_of] * J + np.tile(np.arange(J), P)

    cnt_late = np.bincount(r[late_pairs], minlength=B) if len(late_pairs) \
        else np.zeros(B, np.int64)
    WL = _even(min(int(cnt_late.max()) * J + 2, NE)) if cnt_late.max() > 0 else 0

    # ---- main scatter plane: col -> slot of occ0 entries (incl. singletons)
    is_rep = np.ones(P * J, bool)
    is_rep[oth] = False                    # drop occ1 of 2-groups
    is_rep[oth[swap]] = True               # partner replaces bad-pair rep
    m_bad_entry = badp[pair_of]            # drop all entries of bad pairs
    m0 = is_rep & ~m_bad_entry
    plane = np.full((NCORES, NT, 128, B), -1, np.int16)
    plane[core[m0], t[m0], pp[m0], ec_rot[m0]] = eslot[m0].astype(np.int16)

    # ---- dup passes: source occ0 slot -> occ1 slot
    e_rep, e_oth = eslot[rep_p], eslot[oth_p]
    early = ~late_g
    have_early = bool(early.any())
    planeAe = None
    if have_early:
        planeAe = np.full((NCORES, NT, 128, NE), -1, np.int16)
        planeAe[core[rep_p[early]], t[rep_p[early]], pp[rep_p[early]],
                e_rep[early]] = e_oth[early].astype(np.int16)
    planeAl = None
    have_late = bool(late_g.any()) and WL > 0
    if have_late:
        assert (e_rep[late_g] < WL).all() and (e_oth[late_g] < WL).all()
        planeAl = np.full((NCORES, NT, 128, WL), -1, np.int16)
        planeAl[core[rep_p[late_g]], t[rep_p[late_g]], pp[rep_p[late_g]],
                e_rep[late_g]] = e_oth[late_g].astype(np.int16)

    nmat = n_per_row.reshape(NCORES, NT, 128)
    pairmask = (np.arange(SP)[None, None, None, :] < nmat[..., None]).astype(F16)
    bp = bad_pairs
    pairmask[r[bp] // RPC, (r[bp] % RPC) // 128, r[bp] % 128, srank[bp]] = 0

    return dict(plane=plane, planeAe=planeAe, planeAl=planeAl,
                pairmask=pairmask, SP=SP, NE=NE, WL=WL,
                have_early=have_early, have_late=have_late,
                bad_pairs=bad_pairs)


# ------------------------------------------------------------- device kernel
@with_exitstack
def _build(ctx: ExitStack, tc: "tile.TileContext", io: dict, SP: int, NE: int,
           WL: int, have_early: bool, have_late: bool):
    nc = tc.nc
    y_d, pl_d, pm_d, out_d = io["y8"], io["plane"], io["pm"], io["out"]

    consts = ctx.enter_context(tc.tile_pool(name="consts", bufs=1))
    ones_f32c = consts.tile([128, 1], F32, tag="ones_f32c")
    nc.vector.memset(ones_f32c[:], 1.0)
    wz = consts.tile([128, 2, 512], F8, tag="wz")
    nc.vector.memset(wz[:], 0.0)

    ypool = ctx.enter_context(tc.tile_pool(name="y", bufs=1))
    y = ypool.tile([128, NCH, 2 * NKP, CW], F8, tag="y", name="y")

    gpool = ctx.enter_context(tc.tile_pool(name="gbf", bufs=1))
    gbf = {tt: gpool.tile([128, B], DF16, tag=f"gbf{tt}", name=f"gbf{tt}")
           for tt in range(NT)}
    plpool = ctx.enter_context(tc.tile_pool(name="plane", bufs=1))
    papool = ctx.enter_context(tc.tile_pool(name="passA", bufs=1))
    pae = pal = {}
    if have_early:
        pae = {tt: papool.tile([128, NE], I16, tag=f"pae{tt}", name=f"pae{tt}")
               for tt in range(NT)}
    if have_late:
        pal = {tt: papool.tile([128, WL], I16, tag=f"pal{tt}", name=f"pal{tt}")
               for tt in range(NT)}

    lpool = ctx.enter_context(tc.tile_pool(name="loss", bufs=1))
    pmall = lpool.tile([128, NT, SP], DF16, tag="pmall")

    # ---- DMA: chunk 7 dead last; plane pieces just-in-time
    pl = {}
    def load_piece(pc):
        for tt in range(NT):
            p = plpool.tile([128, PW[pc]], I16, tag=f"pl{tt}_{pc % 2}",
                            name=f"pl{tt}_{pc}")
            pl[(tt, pc)] = p
            nc.sync.dma_start(p[:], pl_d[tt][:, POFF[pc]:POFF[pc] + PW[pc]])

    nc.sync.dma_start(y[:, 0], y_d[0])
    nc.sync.dma_start(y[:, 1], y_d[1])
    load_piece(0)
    nc.sync.dma_start(y[:, 2], y_d[2])
    nc.sync.dma_start(y[:, 3], y_d[3])
    load_piece(1)
    nc.sync.dma_start(y[:, 4], y_d[4])
    nc.sync.dma_start(y[:, 5], y_d[5])
    load_piece(2)
    if have_early:
        for tt in range(NT):
            nc.sync.dma_start(pae[tt][:], io["passAe"][tt])
    if have_late:
        for tt in range(NT):
            nc.sync.dma_start(pal[tt][:], io["passAl"][tt])
    for tt in range(NT):
        nc.sync.dma_start(pmall[:, tt, :], pm_d[tt])
    nc.sync.dma_start(y[:, 6], y_d[6])
    load_piece(3)
    load_piece(4)
    nc.sync.dma_start(y[:, 7], y_d[7])

    dpool = ctx.enter_context(tc.tile_pool(name="dq", bufs=2))
    hpool = ctx.enter_context(tc.tile_pool(name="hacc", bufs=1))
    expool = ctx.enter_context(tc.tile_pool(name="extra", bufs=2))
    elpool = ctx.enter_context(tc.tile_pool(name="elb", bufs=2))
    hacc = {tt: hpool.tile([128, NE], DF16, tag=f"hacc{tt}", name=f"hacc{tt}")
            for tt in range(NT)}

    dq = {}

    def scatter_piece(pc):
        for tt in range(NT):
            d = dpool.tile([128, NE], DF16, tag=f"d{tt}", name=f"d{tt}_{pc}")
            dq[(tt, pc)] = d
            nc.gpsimd.local_scatter(
                d[:], gbf[tt][:, POFF[pc]:POFF[pc] + PW[pc]],
                pl[(tt, pc)][:], 128, NE, PW[pc])

    with tc.tile_pool(name="gpsum", bufs=1, space="PSUM") as gpsum:
        for c in range(NCH):
            for tt in range(NT):
                ps = gpsum.tile([128, CW], F32, tag=f"ps{tt}_{c % 2}",
                                name=f"ps{tt}_{c}")
                for kp in range(NKP):
                    nc.tensor.matmul(
                        ps[:],
                        lhsT=y[:, 0, 2 * kp:2 * kp + 2, tt * 128:(tt + 1) * 128],
                        rhs=y[:, c, 2 * kp:2 * kp + 2, :],
                        start=(kp == 0), stop=(kp == NKP - 1),
                        perf_mode=DR,
                    )
                dst = gbf[tt][:, c * CW:(c + 1) * CW]
                if (c * NT + tt) % 2 == 0:
                    nc.vector.tensor_copy(dst, ps[:])
                else:
                    nc.scalar.copy(dst, ps[:])
            if c == 1:
                scatter_piece(0)
            elif c == 3:
                scatter_piece(1)
                for tt in range(NT):
                    nc.vector.tensor_tensor(hacc[tt][:], dq[(tt, 0)][:],
                                            dq[(tt, 1)][:], ALU.add)
            elif c == 5:
                scatter_piece(2)
                for tt in range(NT):
                    nc.vector.tensor_tensor(hacc[tt][:], hacc[tt][:],
                                            dq[(tt, 2)][:], ALU.add)
            elif c == 6:
                # piece 3 (chunk 6 columns) + early dup pass, pre-chunk-7
                scatter_piece(3)
                for tt in range(NT):
                    nc.vector.tensor_tensor(hacc[tt][:], hacc[tt][:],
                                            dq[(tt, 3)][:], ALU.add)
                if have_early:
                    for tt in range(NT):
                        e = expool.tile([128, NE], DF16, tag=f"eAe{tt % 2}",
                                        name=f"eAe{tt}")
                        nc.gpsimd.local_scatter(e[:], hacc[tt][:], pae[tt][:],
                                                128, NE, NE)
                        nc.vector.tensor_tensor(hacc[tt][:], hacc[tt][:],
                                                e[:], ALU.add)
                # p-state bridge: keep PE warm until chunk 7 lands
                wps = gpsum.tile([128, CW], F32, tag="ps0_1", name="warm")
                for i in range(NWARM):
                    nc.tensor.matmul(wps[:], lhsT=wz[:, :, 0:128],
                                     rhs=wz[:], start=True, stop=True,
                                     perf_mode=DR)

        # ---- tail: piece 4 scatters, late pass, exp/den/lse per tile
        scatter_piece(4)

        denall = lpool.tile([128, NT, SP], F32, tag="denall")
        accq = lpool.tile([128, NT], F32, tag="accq")
        for tt in range(NT):
            nc.vector.tensor_tensor(hacc[tt][:], hacc[tt][:], dq[(tt, 4)][:],
                                    ALU.add)
            if have_late:
                e = expool.tile([128, WL], DF16, tag=f"eAl{tt % 2}",
                                name=f"eAl{tt}")
                nc.gpsimd.local_scatter(e[:], hacc[tt][:, 0:WL], pal[tt][:],
                                        128, WL, WL)
                nc.vector.tensor_tensor(hacc[tt][:, 0:WL], hacc[tt][:, 0:WL],
                                        e[:], ALU.add)
            ebuf = elpool.tile([128, NE], F32, tag="ebuf")
            nc.scalar.activation(ebuf[:], hacc[tt][:], AF.Exp, scale=KSC)
            e3 = ebuf[:, 0:SP * J].rearrange("p (s j) -> p s j", j=J)
            nc.vector.tensor_reduce(denall[:, tt, :], e3, AX.X, ALU.add)
            lnd = elpool.tile([128, SP], F32, tag="lnd")
            nc.scalar.activation(lnd[:], denall[:, tt, :], AF.Ln)
            l0 = hacc[tt][:, 0:SP * J].rearrange("p (s j) -> p s j", j=J)[:, :, 0]
            diff = elpool.tile([128, SP], F32, tag="diff")
            nc.vector.scalar_tensor_tensor(diff[:], l0, -KSC, lnd[:],
                                           ALU.mult, ALU.add)
            scrap = elpool.tile([128, SP], F32, tag="scrap")
            nc.vector.scalar_tensor_tensor(
                scrap[:], diff[:], 1.0, pmall[:, tt, :], ALU.mult, ALU.mult,
                accum_out=accq[:, tt:tt + 1])

    with tc.tile_pool(name="p5psum", bufs=1, space="PSUM") as p5psum:
        acc1 = lpool.tile([128, 1], F32, tag="acc1")
        nc.vector.tensor_reduce(acc1[:], accq[:], AX.X, ALU.add)
        ps = p5psum.tile([1, 1], F32, tag="ps_out")
        nc.tensor.matmul(ps[:], lhsT=acc1[:], rhs=ones_f32c[:, 0:1],
                         start=True, stop=True)
        res = lpool.tile([1, 1], F32, tag="res")
        nc.scalar.copy(res[:], ps[:])
        nc.sync.dma_start(out_d[:], res[:])


def build_nc(SP, NE, WL, have_early, have_late, enable_asserts=False):
    nc = bacc.Bacc("TRN2", target_bir_lowering=False, debug=False,
                   enable_asserts=enable_asserts, num_devices=NCORES)
    io = {
        "y8": nc.dram_tensor("y8", [NCH, 128, 2 * NKP, CW], F8,
                             kind="ExternalInput").ap(),
        "plane": nc.dram_tensor("plane", [NT, 128, B], I16,
                                kind="ExternalInput").ap(),
        "pm": nc.dram_tensor("pm", [NT, 128, SP], DF16,
                             kind="ExternalInput").ap(),
        "out": nc.dram_tensor("out", [1, 1], F32, kind="ExternalOutput").ap(),
    }
    if have_early:
        io["passAe"] = nc.dram_tensor("passAe", [NT, 128, NE], I16,
                                      kind="ExternalInput").ap()
    if have_late:
        io["passAl"] = nc.dram_tensor("passAl", [NT, 128, WL], I16,
                                      kind="ExternalInput").ap()
    with tile.TileContext(nc) as tc:
        _build(tc, io, SP, NE, WL, have_early, have_late)
    nc.compile()
    return nc


def _normalize(x):
    x = np.asarray(x, np.float32)
    w = np.sqrt((x.astype(np.float64) ** 2).sum(axis=1, keepdims=True))
    w = np.maximum(w, 1e-8)
    return (x / w).astype(np.float32)


def make_in_maps(x, plan):
    xn = _normalize(x)
    x8 = np.clip(xn * S, -240.0, 240.0).astype(FP8)
    in_maps = []
    for k in range(NCORES):
        xr = np.roll(x8, -RPC * k, axis=0)                     # [B, D]
        y8 = xr.T.reshape(2 * NKP, 128, B).transpose(1, 0, 2)  # [128, 16, B]
        y8c = np.ascontiguousarray(
            y8.reshape(128, 2 * NKP, NCH, CW).transpose(2, 0, 1, 3))
        m = {
            "y8": y8c,
            "plane": plan["plane"][k],
            "pm": plan["pairmask"][k],
        }
        if plan["have_early"]:
            m["passAe"] = plan["planeAe"][k]
        if plan["have_late"]:
            m["passAl"] = plan["planeAl"][k]
        in_maps.append(m)
    return in_maps


def host_fixup(x, anchor_idx, pos_idx, neg_idx, bad_pairs):
    """Exact loss terms for pairs masked out on the device."""
    if len(bad_pairs) == 0:
        return 0.0
    xn = _normalize(x).astype(np.float64)
    a = anchor_idx[bad_pairs]
    cols = np.concatenate([pos_idx[bad_pairs][:, None], neg_idx[bad_pairs]],
                          axis=1)
    logits = np.einsum("pd,pjd->pj", xn[a], xn[cols]) / TEMP
    mx = logits.max(axis=1, keepdims=True)
    lse = np.log(np.exp(logits - mx).sum(axis=1)) + mx[:, 0]
    return float((lse - logits[:, 0]).sum())


def kernel(**inputs):
    x = np.asarray(inputs["x"], np.float32)
    anchor_idx = np.asarray(inputs["anchor_idx"])
    pos_idx = np.asarray(inputs["pos_idx"])
    neg_idx = np.asarray(inputs["neg_idx"])
    P = anchor_idx.shape[0]

    plan = build_plan(anchor_idx, pos_idx, neg_idx)
    nc = build_nc(plan["SP"], plan["NE"], plan["WL"],
                  plan["have_early"], plan["have_late"])
    in_maps = make_in_maps(x, plan)
    res = run_bass_kernel_spmd(nc, in_maps, list(range(NCORES)))
    total = sum(float(res.results[k]["out"][0, 0]) for k in range(NCORES))
    total += host_fixup(x, anchor_idx, pos_idx, neg_idx, plan["bad_pairs"])
    return np.float32(total / P)
